# revision 30
# baseline (speedup 1.0000x reference)
"""FUSE bucket-model scan kernel for Trainium2 (8 NeuronCores) — all-DVE scan.

Strategy
--------
H=4096 HRUs sharded across 8 cores (512 each) as [128 partitions x 4 groups];
the two bucket states are packed into one [128, 8] tile (cols 0-3: upper zone
per group, cols 4-7: lower zone). The T=8192 time recurrence is a sequential
scan; all forcing-derived operand tiles (Z = [pn | bn], A' = [1-(pet+perc)/m1
| perc/m2]) are precomputed on the host in a scan-friendly [128, T, 16]
layout and streamed per 128-step chunk.

The 12.3ms baseline ran the power nonlinearity (r^a = exp(a ln r)) through
the Activation engine: two transcendental round-trips + cross-engine hops
gave a ~1.5us/step loop-carried chain. Here the whole recurrence lives on
the Vector engine (DVE) as fused custom multi-ALU-stage ops (each op costs
the same ~69ns exec + ~95ns result latency regardless of its up-to-8 ALU
stages), and the loop-carried cycle is squeezed to 4 dependent hops:

  LOGE  = (float(int32_view(r))*2^-23 - C) * a/16          [3 ALU ops]
  LOGP  = (P3(mu) - mu) * a/16,  mu = bits(r)|bits(1.0)    [8 ALU ops]
          (both depend only on r -> run back-to-back at cycle start;
           P3 is a deg-3 fit of log2 on [1,2], exponent-field bit tricks
           give the rest; a/16 = sigmoid-params exponent, host-prescaled)
  EXPB  = u = 1 + v*P2(v), v = max(LOGE+LOGP, -1)          [8 ALU ops]
  EXP3B = hD = ((col<4) - u^16) * Z                        [8 ALU ops]
          = [pn*(1-x1) | -bn*x2], streamed out for host runoff
  RCLIPA= r' = clip(phi + hD, eps, 1)                      [3 ALU ops]

phi = [r1*A'1 | r1*pc12 + r2] is built by two stock DVE ops placed so each
fills one result-latency gap of the chain (zero interference, verified in
the timeline trace: exactly 725ns/step steady-state, no semaphores). The
chunk stream pre-touch runs on the otherwise idle Activation engine; h-D
goes straight to a chunk buffer DMA-d out, and the host computes
runoff = p - (hD1*m1 + hD2*m2), avoiding any cross-engine consumer of
DVE tiles (which would cost a SEQ-blocking extra semaphore wait).

State is normalized r = clip(s/maxwatr, eps, 1); carrying r instead of
(sigma, r) loses at most eps/step. End-to-end vs the f64 reference:
rel-of-max ~1.3e-3 (tolerance 2e-2), bit-exact with the numpy emulation
of the DVE pipeline. Model-estimated device time ~6.14ms (2.0x baseline).
"""
import numpy as np

import concourse.bass as bass
import concourse.bacc as bacc
import concourse.mybir as mybir
from concourse.bass import ds
from concourse.tile import TileContext
from concourse.bass_utils import run_bass_kernel_spmd

F32 = mybir.dt.float32
I32 = mybir.dt.int32
AF = mybir.ActivationFunctionType
OP = mybir.AluOpType

# --- custom fused DVE ops ---------------------------------------------------
from concourse.dve_spec import Spec, Src0, Src1, maxx, minn, Idx, lower as _dve_lower
from concourse.dve_spec import C0 as _C0, C1 as _C1, C2 as _C2, One as _One, Zero as _Zero
from concourse.dve_spec import Bin, AluOp
from concourse import dve_ops as _dvo
from concourse.dve_uop import DveOpSpec as _DveOpSpec


def _register_custom_op(name, spec):
    for op in _dvo.OPS:
        if op.name == name:
            return op
    row = _dvo._CUSTOM_DVE_ROW_BASE + len(_dvo.OPS)
    _dvo._SUB_OPCODE_FOR_NAME[name] = row
    shas = {}
    for ver in ("v3", "v4"):
        try:
            uops = _dve_lower(spec, ver=ver)
            shas[ver] = _DveOpSpec(name=name, opcode=row, uops=uops,
                                   rd1_en=True).sha(ver)
        except Exception:
            pass
    op = _dvo.DveOp(name, spec, subdim=False, uops_sha=shas)
    _dvo.OPS.append(op)
    _dvo.CUSTOM_DVE_SPECS[name] = spec
    return op


# Polynomial coefficients (Chebyshev LS fits).
# log2(mu) on [1,2], deg 3 (c0 folded into the LOGE bias constant):
LC = [-2.1338165360123584, 3.0107302949770647, -1.0294927543909727,
      0.15391353466591032]
# (2^v - 1)/v on [-1,0], deg 2  =>  2^v ~= 1 + v*(EC0 + v*(EC1 + v*EC2)):
EC = [0.6927658142758559, 0.23552592911390743, 0.043112826547017494]
C23 = float(2.0 ** -23)
CL1 = float(126.0 - LC[0])

_f = lambda x: np.asarray(x, np.float32)


def _mu_of(in1):
    return (np.asarray(in1, np.float32).view(np.int32)
            | np.int32(0x3F800000)).view(np.float32)


def _ref_loge(in0, in1, s0, s1, imm2):
    # in0 = int32 view of r; in1 = PW/16. out = (float(Iv)*2^-23 - CL1)*a/16
    f = in0.astype(np.float32)
    y0 = _f(_f(f * np.float32(s0)) - np.float32(s1))
    return _f(y0 * in1.astype(np.float32))


FUSE_LOGE = _register_custom_op(
    "FUSE_LOGE",
    Spec(body=(Src0 * _C0 - _C1) * Src1, reference=_ref_loge),
)


def _ref_logp(in0, in1, s0, s1, imm2):
    # in0 = r (f32); in1 = PW/16. out = (P3(mu) - mu)*a/16
    mu = _mu_of(in0)
    E = _f(_f(_f(_f(_f(mu * np.float32(s0)) + np.float32(s1)) * mu)
              + np.float32(imm2)) * mu)
    return _f(_f(E - mu) * in1.astype(np.float32))


_mu1 = Bin(AluOp.BITWISE_OR, Src0, _One)
FUSE_LOGP = _register_custom_op(
    "FUSE_LOGP",
    Spec(body=((((_mu1 * _C0 + _C1) * _mu1 + _C2) * _mu1) - _mu1) * Src1,
         reference=_ref_logp),
)


def _ref_expb(in0, in1, s0, s1, imm2):
    # v = max(loge_part + logp_part, -1); u = 1 + v*(EC0 + v*(EC1 + v*EC2))
    y = _f(in0.astype(np.float32) + in1.astype(np.float32))
    v = np.maximum(y, np.float32(-1.0))
    q = _f(_f(_f(_f(_f(v * np.float32(imm2)) + np.float32(s1)) * v)
              + np.float32(s0)) * v)
    return _f(q + np.float32(1.0))


_v = maxx(Src0 + Src1, _Zero - _One)
_q = ((_v * _C2 + _C1) * _v + _C0) * _v
FUSE_EXPB = _register_custom_op(
    "FUSE_EXPB",
    Spec(body=_q + _One, reference=_ref_expb),
)


def _ref_exp3b(in0, in1, s0, s1, imm2):
    # hD = (beta - u^16) * Z, beta = 1 for cols < s0 (upper zone), else 0:
    # left half gives pn*(1 - x1), right half gives -bn*x2.
    u = in0.astype(np.float32)
    u2 = _f(u * u)
    u4 = _f(u2 * u2)
    u8 = _f(u4 * u4)
    u16 = _f(u8 * u8)
    n = in0.shape[-1]
    beta = (np.arange(n, dtype=np.float32) < np.float32(s0)).astype(np.float32)
    return _f(_f(beta - u16) * in1.astype(np.float32))


_su2 = Src0 * Src0
_su4 = _su2 * _su2
_su8 = _su4 * _su4
_su16 = _su8 * _su8
FUSE_EXP3B = _register_custom_op(
    "FUSE_EXP3B",
    Spec(body=((Idx < _C0) - _su16) * Src1, reference=_ref_exp3b),
)


def _ref_clipa(in0, in1, s0, s1, imm2):
    return np.clip(_f(in0.astype(np.float32) + in1.astype(np.float32)),
                   np.float32(s0), np.float32(1.0))


FUSE_RCLIPA = _register_custom_op(
    "FUSE_RCLIPA",
    Spec(body=minn(maxx(Src0 + Src1, _C0), _One), reference=_ref_clipa),
)

T = 8192
H = 4096
NCORES = 8
HC = H // NCORES          # 512 HRUs per core
P = 128                   # partitions
G = HC // P               # 4 groups
K = 128                   # timesteps per chunk
EPS = 1e-6


def build_nc(t_total=T, k_chunk=K, unrolled=False):
    nc = bacc.Bacc()
    ZAt = nc.dram_tensor("ZA", [P, t_total * 16], F32, kind="ExternalInput")
    Ct = nc.dram_tensor("CONSTS", [P, 24], F32, kind="ExternalInput")
    # raw h = r^a * [pn | bn] streamed out; host folds maxwatr + pair-sum
    HO = nc.dram_tensor("HO", [P, t_total * 8], F32, kind="ExternalOutput")

    n_chunks = t_total // k_chunk
    with TileContext(nc) as tc:
        with (
            tc.tile_pool(name="const", bufs=1) as cpool,
            tc.tile_pool(name="zin", bufs=3) as zpool,
            tc.tile_pool(name="rout", bufs=3) as ropool,
            tc.tile_pool(name="work", bufs=4) as wpool,
        ):
            cst_in = cpool.tile([P, 24], F32)
            cst = cpool.tile([P, 24], F32)
            nc.sync.dma_start(out=cst_in[:], in_=Ct[:])
            nc.vector.tensor_copy(out=cst[:], in_=cst_in[:])
            pw = cst[:, 8:16]
            rrt = cpool.tile([P, 8], F32)
            # state: r = clip(s/maxwatr, eps, 1); all scan ops live on DVE
            nc.vector.tensor_scalar(out=rrt[:], in0=cst[:, 0:8], scalar1=EPS,
                                    scalar2=None, op0=OP.max)

            import contextlib
            def chunk_iter():
                if unrolled:
                    for i in range(n_chunks):
                        yield contextlib.nullcontext(i)
                else:
                    yield tc.For_i(0, n_chunks, staggered_reset=True,
                                   hint_engines=(mybir.EngineType.DVE,
                                                 mybir.EngineType.Pool))
            for _cm in chunk_iter():
              with _cm as ci:
                  zc = zpool.tile([P, k_chunk * 16], F32)
                  hc = ropool.tile([P, k_chunk * 8], F32)
                  nc.sync.dma_start(out=zc[:], in_=ZAt[:, ds(ci * (k_chunk * 16), k_chunk * 16)])
                  # pre-touch on the otherwise-idle Activation engine: absorbs
                  # the DMA-completion wait so per-step stream readers only
                  # ever wait on one compute semaphore.
                  zc2 = zpool.tile([P, k_chunk * 16], F32, tag="zc2")
                  nc.scalar.activation(zc2[:], zc[:], AF.Copy)

                  for k in range(k_chunk):
                      z8 = zc2[:, k * 16:k * 16 + 8]
                      a8 = zc2[:, k * 16 + 8:k * 16 + 16]
                      et = wpool.tile([P, 8], F32, tag="et")
                      pt = wpool.tile([P, 8], F32, tag="pt")
                      u = wpool.tile([P, 8], F32, tag="u")
                      h = hc[:, k * 8:k * 8 + 8]
                      phi = wpool.tile([P, 8], F32, tag="phi")

                      # 4-hop dependent DVE chain (~164ns/hop): the two log
                      # halves both depend only on r, so they run back-to-back
                      # at the cycle start and EXPB merges them; W and PHIR
                      # fill the result-drain gaps.
                      # exponent half: (float(int32(r))*2^-23 - CL1) * a/16
                      nc.vector._custom_dve(FUSE_LOGE, out=et[:],
                                            in0=rrt[:].bitcast(I32), in1=pw,
                                            s0=C23, s1=CL1)
                      # mantissa half: (P3(mu) - mu) * a/16, mu = bits(r)|1.0
                      nc.vector._custom_dve(FUSE_LOGP, out=pt[:],
                                            in0=rrt[:], in1=pw,
                                            s0=LC[3], s1=LC[2], imm2=LC[1])
                      # filler 1: phi = r1 * [A'1 | pc12] (r1 broadcast)
                      r1b = rrt[:, 0:4].rearrange('p (o f) -> p o f', o=1) \
                                       .broadcast_to([P, 2, 4])
                      a3 = a8.rearrange('p (o f) -> p o f', o=2)
                      w3 = phi[:].rearrange('p (o f) -> p o f', o=2)
                      nc.vector.tensor_tensor(out=w3, in0=r1b, in1=a3, op=OP.mult)
                      # filler 2: phi_r += r2 (in place)
                      nc.vector.tensor_tensor(out=phi[:, 4:8], in0=phi[:, 4:8],
                                              in1=rrt[:, 4:8], op=OP.add)
                      # u = 1 + v*(EC0 + v*(EC1 + v*EC2)), v = max(et+pt, -1)
                      nc.vector._custom_dve(FUSE_EXPB, out=u[:],
                                            in0=et[:], in1=pt[:],
                                            s0=EC[0], s1=EC[1], imm2=EC[2])
                      # hD = ((col<4) - u^16) * Z = [pn*(1-x1) | -bn*x2],
                      # written straight into the chunk output buffer
                      # (streamed out; host turns it into runoff)
                      nc.vector._custom_dve(FUSE_EXP3B, out=h,
                                            in0=u[:], in1=z8, s0=4.0)
                      # r' = clip(phi + hD, eps, 1)
                      nc.vector._custom_dve(FUSE_RCLIPA, out=rrt[:],
                                            in0=phi[:], in1=h, s0=EPS)

                  nc.sync.dma_start(
                      out=HO[:, ds(ci * (k_chunk * 8), k_chunk * 8)],
                      in_=hc[:])
    nc.compile()
    return nc


def _host_prepare(forcing, initial_state, raw_params, param_lower, param_upper,
                  t_total=T):
    """Derive per-core input arrays. All fp32, same op order as the device."""
    f32 = np.float32
    lo = param_lower.astype(f32)
    hi = param_upper.astype(f32)
    sg = (1.0 / (1.0 + np.exp(-raw_params.astype(np.float64))))
    phys = (lo.astype(np.float64) + (hi - lo).astype(np.float64) * sg).astype(f32)
    mw1, mw2, percrte, baserte, qbp, axv = [phys[:, i].copy() for i in range(6)]
    inv1 = (f32(1.0) / mw1).astype(f32)
    inv2 = (f32(1.0) / mw2).astype(f32)

    p_r = forcing[:, :, 0].astype(f32)    # [T, H]
    pet = forcing[:, :, 1].astype(f32)

    pn = (p_r * inv1[None, :]).astype(f32)
    ap1 = (f32(1.0) - ((pet + percrte[None, :]) * inv1[None, :])).astype(f32)
    bn = (baserte * inv2).astype(f32)
    pc12 = (percrte * inv2).astype(f32)

    s1n = np.clip(initial_state[:, 0].astype(f32) * inv1, EPS, 1.0).astype(f32)
    s2n = np.clip(initial_state[:, 1].astype(f32) * inv2, EPS, 1.0).astype(f32)

    in_maps = []
    for c in range(NCORES):
        sl = slice(c * HC, (c + 1) * HC)
        # [T, HC] -> [T, G, P] -> [P, T, G]
        def tg(a):
            return np.ascontiguousarray(
                a[:, sl].reshape(t_total, G, P).transpose(2, 0, 1))
        ZA = np.empty((P, t_total, 16), f32)
        ZA[:, :, 0:4] = tg(pn)
        ZA[:, :, 4:8] = bn[sl].reshape(G, P).T[:, None, :]
        ZA[:, :, 8:12] = tg(ap1)
        ZA[:, :, 12:16] = pc12[sl].reshape(G, P).T[:, None, :]

        def pk(a1, a2):
            out = np.empty((P, 8), f32)
            out[:, 0:4] = a1[sl].reshape(G, P).T
            out[:, 4:8] = a2[sl].reshape(G, P).T
            return out

        # exponent tile pre-divided by 16 (the exp2 range reduction)
        consts = np.concatenate([pk(s1n, s2n),
                                 pk(axv / np.float32(16.0),
                                    qbp / np.float32(16.0)),
                                 pk(mw1, mw2)], axis=1)
        in_maps.append({
            "ZA": ZA.reshape(P, t_total * 16),
            "CONSTS": consts,
        })
    return in_maps, (mw1, mw2)


_NC_CACHE = {}


def kernel(forcing, initial_state, raw_params, param_lower, param_upper):
    forcing = np.asarray(forcing)
    initial_state = np.asarray(initial_state)
    raw_params = np.asarray(raw_params)
    param_lower = np.asarray(param_lower)
    param_upper = np.asarray(param_upper)
    t_total = forcing.shape[0]
    if t_total not in _NC_CACHE:
        _NC_CACHE[t_total] = build_nc(t_total=t_total)
    nc = _NC_CACHE[t_total]
    in_maps, (mw1, mw2) = _host_prepare(forcing, initial_state, raw_params,
                                        param_lower, param_upper,
                                        t_total=t_total)
    res = run_bass_kernel_spmd(nc, in_maps, core_ids=list(range(NCORES)))
    # per-core HO: [P, T, 8]; cols g / 4+g hold hD = (pn*(1-x1), -bn*x2) of
    # HRU g*P+p; runoff = qsx + qb = p - (hD1*m1 + hD2*m2)
    p_raw = forcing[:, :, 0].astype(np.float32)    # [T, H]
    out = np.empty((t_total, H), np.float32)
    for c in range(NCORES):
        sl = slice(c * HC, (c + 1) * HC)
        ho = res.results[c]["HO"].reshape(P, t_total, 8)
        m1c = mw1[sl].reshape(G, P)
        m2c = mw2[sl].reshape(G, P)
        for g in range(G):
            hd1 = ho[:, :, g]                # [P, T]
            hd2 = ho[:, :, 4 + g]
            cols = slice(c * HC + g * P, c * HC + (g + 1) * P)
            out[:, cols] = p_raw[:, cols] - (
                hd1 * m1c[g][:, None] + hd2 * m2c[g][:, None]).T
    return out


# revision 35
# speedup vs baseline: 1.1367x; 1.1367x over previous
"""FUSE bucket-model scan kernel for Trainium2 (8 NeuronCores) — all-DVE scan.

Strategy
--------
H=4096 HRUs sharded across 8 cores (512 each) as [128 partitions x 4 groups];
the two bucket states are packed into one [128, 8] tile (cols 0-3: upper zone
per group, cols 4-7: lower zone). The T=8192 time recurrence is a sequential
scan; all forcing-derived operand tiles (Z = [pn | bn], A' = [1-(pet+perc)/m1
| perc/m2]) are precomputed on the host in a scan-friendly [128, T, 16]
layout and streamed per 128-step chunk.

The 12.3ms baseline ran the power nonlinearity (r^a = exp(a ln r)) through
the Activation engine: two transcendental round-trips + cross-engine hops
gave a ~1.5us/step loop-carried chain. Here the whole recurrence lives on
the Vector engine (DVE) as fused custom multi-ALU-stage ops (each op costs
the same ~69ns exec + ~95ns result latency regardless of its up-to-8 ALU
stages), and the loop-carried cycle is squeezed to 4 dependent nodes:

  LOGF  = lam = log2(r) ~= y0 + mu + c2*mu^2               [7 ALU ops]
          y0 = float(int32_view(r))*2^-23 - (126-c0);
          mu = bits(r)|bits(1.0) in [1,2).  Uses the c1=2-CONSTRAINED
          quadratic fit of log2 on [1,2] so that the exponent part's -mu
          and the poly's +2mu merge, fitting the whole log in one node.
  EXPAF = u = 1 + v*P2(v), v = max(lam*a/16, -1)           [8 ALU ops]
          (a/16 = sigmoid-param exponents, host-prescaled)
  EXP3B = hD = ((col<4) - u^16) * Z                        [8 ALU ops]
          = [pn*(1-x1) | -bn*x2], streamed out for host runoff
  RCLIPA= r' = clip(phi + hD, eps, 1)                      [3 ALU ops]

phi = [r1*A'1 | r1*pc12 + r2] is built by two stock DVE ops placed so each
fills one result-latency gap of the chain (zero interference, verified in
the timeline trace: exactly 656ns/step steady-state, no semaphores). The
chunk stream pre-touch runs on the otherwise idle Activation engine; h-D
goes straight to a chunk buffer DMA-d out, and the host computes
runoff = p - (hD1*m1 + hD2*m2), avoiding any cross-engine consumer of
DVE tiles (which would cost a SEQ-blocking extra semaphore wait).

State is normalized r = clip(s/maxwatr, eps, 1); carrying r instead of
(sigma, r) loses at most eps/step. End-to-end vs the f64 reference:
rel-of-max ~7.8e-3 (tolerance 2e-2), bit-exact with the numpy emulation
of the DVE pipeline. Model-estimated device time ~5.57ms (2.2x baseline).
"""
import numpy as np

import concourse.bass as bass
import concourse.bacc as bacc
import concourse.mybir as mybir
from concourse.bass import ds
from concourse.tile import TileContext
from concourse.bass_utils import run_bass_kernel_spmd

F32 = mybir.dt.float32
I32 = mybir.dt.int32
AF = mybir.ActivationFunctionType
OP = mybir.AluOpType

# --- custom fused DVE ops ---------------------------------------------------
from concourse.dve_spec import Spec, Src0, Src1, maxx, minn, Idx, lower as _dve_lower
from concourse.dve_spec import C0 as _C0, C1 as _C1, C2 as _C2, One as _One, Zero as _Zero
from concourse.dve_spec import Bin, AluOp
from concourse import dve_ops as _dvo
from concourse.dve_uop import DveOpSpec as _DveOpSpec


def _register_custom_op(name, spec):
    for op in _dvo.OPS:
        if op.name == name:
            return op
    row = _dvo._CUSTOM_DVE_ROW_BASE + len(_dvo.OPS)
    _dvo._SUB_OPCODE_FOR_NAME[name] = row
    shas = {}
    for ver in ("v3", "v4"):
        try:
            uops = _dve_lower(spec, ver=ver)
            shas[ver] = _DveOpSpec(name=name, opcode=row, uops=uops,
                                   rd1_en=True).sha(ver)
        except Exception:
            pass
    op = _dvo.DveOp(name, spec, subdim=False, uops_sha=shas)
    _dvo.OPS.append(op)
    _dvo.CUSTOM_DVE_SPECS[name] = spec
    return op


# Polynomial coefficients.
# log2(mu) on [1,2] as c0 + 2*mu + c2*mu^2 (c1 FIXED at 2, Lawson-minimax):
# with e' = y0 - mu, lam = e' + c0 + 2mu + c2mu^2 = y0 + c0 + mu + c2*mu^2 —
# the -mu/+mu cancellation lets the whole log fit ONE 7-op DVE node.
LC0 = -1.6587157930680616
LC2 = -0.33582938191046763
# (2^v - 1)/v on [-1,0], deg 2  =>  2^v ~= 1 + v*(EC0 + v*(EC1 + v*EC2)):
EC = [0.6927658142758559, 0.23552592911390743, 0.043112826547017494]
C23 = float(2.0 ** -23)
CL1 = float(126.0 - LC0)

_f = lambda x: np.asarray(x, np.float32)


def _mu_of(in1):
    return (np.asarray(in1, np.float32).view(np.int32)
            | np.int32(0x3F800000)).view(np.float32)


def _ref_logf(in0, in1, s0, s1, imm2):
    # in0 = int32 view of r; in1 = r (f32).
    # lam = (float(Iv)*2^-23 - CL1) + mu + c2*mu^2
    f = in0.astype(np.float32)
    y0 = _f(_f(f * np.float32(s0)) - np.float32(s1))
    mu = _mu_of(in1)
    return _f(_f(y0 + mu) + _f(_f(mu * mu) * np.float32(imm2)))


_muf = Bin(AluOp.BITWISE_OR, Src1, _One)
FUSE_LOGF = _register_custom_op(
    "FUSE_LOGF",
    Spec(body=((Src0 * _C0 - _C1) + _muf) + (_muf * _muf) * _C2,
         reference=_ref_logf),
)


def _ref_expaf(in0, in1, s0, s1, imm2):
    # v = max(lam * a/16, -1); u = 1 + v*(EC0 + v*(EC1 + v*EC2))
    y = _f(in0.astype(np.float32) * in1.astype(np.float32))
    v = np.maximum(y, np.float32(-1.0))
    q = _f(_f(_f(_f(_f(v * np.float32(imm2)) + np.float32(s1)) * v)
              + np.float32(s0)) * v)
    return _f(q + np.float32(1.0))


_v = maxx(Src0 * Src1, _Zero - _One)
_q = ((_v * _C2 + _C1) * _v + _C0) * _v
FUSE_EXPAF = _register_custom_op(
    "FUSE_EXPAF",
    Spec(body=_q + _One, reference=_ref_expaf),
)


def _ref_exp3b(in0, in1, s0, s1, imm2):
    # hD = (beta - u^16) * Z, beta = 1 for cols < s0 (upper zone), else 0:
    # left half gives pn*(1 - x1), right half gives -bn*x2.
    u = in0.astype(np.float32)
    u2 = _f(u * u)
    u4 = _f(u2 * u2)
    u8 = _f(u4 * u4)
    u16 = _f(u8 * u8)
    n = in0.shape[-1]
    beta = (np.arange(n, dtype=np.float32) < np.float32(s0)).astype(np.float32)
    return _f(_f(beta - u16) * in1.astype(np.float32))


_su2 = Src0 * Src0
_su4 = _su2 * _su2
_su8 = _su4 * _su4
_su16 = _su8 * _su8
FUSE_EXP3B = _register_custom_op(
    "FUSE_EXP3B",
    Spec(body=((Idx < _C0) - _su16) * Src1, reference=_ref_exp3b),
)


def _ref_clipa(in0, in1, s0, s1, imm2):
    return np.clip(_f(in0.astype(np.float32) + in1.astype(np.float32)),
                   np.float32(s0), np.float32(1.0))


FUSE_RCLIPA = _register_custom_op(
    "FUSE_RCLIPA",
    Spec(body=minn(maxx(Src0 + Src1, _C0), _One), reference=_ref_clipa),
)

T = 8192
H = 4096
NCORES = 8
HC = H // NCORES          # 512 HRUs per core
P = 128                   # partitions
G = HC // P               # 4 groups
K = 128                   # timesteps per chunk
EPS = 1e-6


def build_nc(t_total=T, k_chunk=K, unrolled=False):
    nc = bacc.Bacc()
    ZAt = nc.dram_tensor("ZA", [P, t_total * 16], F32, kind="ExternalInput")
    Ct = nc.dram_tensor("CONSTS", [P, 24], F32, kind="ExternalInput")
    # raw h = r^a * [pn | bn] streamed out; host folds maxwatr + pair-sum
    HO = nc.dram_tensor("HO", [P, t_total * 8], F32, kind="ExternalOutput")

    n_chunks = t_total // k_chunk
    with TileContext(nc) as tc:
        with (
            tc.tile_pool(name="const", bufs=1) as cpool,
            tc.tile_pool(name="zin", bufs=3) as zpool,
            tc.tile_pool(name="rout", bufs=3) as ropool,
            tc.tile_pool(name="work", bufs=4) as wpool,
        ):
            cst_in = cpool.tile([P, 24], F32)
            cst = cpool.tile([P, 24], F32)
            nc.sync.dma_start(out=cst_in[:], in_=Ct[:])
            nc.vector.tensor_copy(out=cst[:], in_=cst_in[:])
            pw = cst[:, 8:16]
            rrt = cpool.tile([P, 8], F32)
            # state: r = clip(s/maxwatr, eps, 1); all scan ops live on DVE
            nc.vector.tensor_scalar(out=rrt[:], in0=cst[:, 0:8], scalar1=EPS,
                                    scalar2=None, op0=OP.max)

            import contextlib
            def chunk_iter():
                if unrolled:
                    for i in range(n_chunks):
                        yield contextlib.nullcontext(i)
                else:
                    yield tc.For_i(0, n_chunks, staggered_reset=True,
                                   hint_engines=(mybir.EngineType.DVE,
                                                 mybir.EngineType.Pool))
            for _cm in chunk_iter():
              with _cm as ci:
                  zc = zpool.tile([P, k_chunk * 16], F32)
                  hc = ropool.tile([P, k_chunk * 8], F32)
                  nc.sync.dma_start(out=zc[:], in_=ZAt[:, ds(ci * (k_chunk * 16), k_chunk * 16)])
                  # pre-touch on the otherwise-idle Activation engine: absorbs
                  # the DMA-completion wait so per-step stream readers only
                  # ever wait on one compute semaphore.
                  zc2 = zpool.tile([P, k_chunk * 16], F32, tag="zc2")
                  nc.scalar.activation(zc2[:], zc[:], AF.Copy)

                  for k in range(k_chunk):
                      z8 = zc2[:, k * 16:k * 16 + 8]
                      a8 = zc2[:, k * 16 + 8:k * 16 + 16]
                      lam = wpool.tile([P, 8], F32, tag="lam")
                      u = wpool.tile([P, 8], F32, tag="u")
                      h = hc[:, k * 8:k * 8 + 8]
                      phi = wpool.tile([P, 8], F32, tag="phi")

                      # 4-node dependent DVE chain (~164ns/hop, 656ns/step):
                      # LOGF -> EXPAF -> EXP3B -> RCLIPA; W and PHIR fill the
                      # result-drain gaps.
                      # lam = log2(r) in ONE node: y0 + mu + c2*mu^2
                      # (c1=2-constrained fit; -mu from the exponent part and
                      # +2mu from the poly merge into a single +mu)
                      nc.vector._custom_dve(FUSE_LOGF, out=lam[:],
                                            in0=rrt[:].bitcast(I32), in1=rrt[:],
                                            s0=C23, s1=CL1, imm2=LC2)
                      # filler 1: phi = r1 * [A'1 | pc12] (r1 broadcast)
                      r1b = rrt[:, 0:4].rearrange('p (o f) -> p o f', o=1) \
                                       .broadcast_to([P, 2, 4])
                      a3 = a8.rearrange('p (o f) -> p o f', o=2)
                      w3 = phi[:].rearrange('p (o f) -> p o f', o=2)
                      nc.vector.tensor_tensor(out=w3, in0=r1b, in1=a3, op=OP.mult)
                      # u = 1 + v*(EC0 + v*(EC1 + v*EC2)), v = max(lam*a/16, -1)
                      nc.vector._custom_dve(FUSE_EXPAF, out=u[:],
                                            in0=lam[:], in1=pw,
                                            s0=EC[0], s1=EC[1], imm2=EC[2])
                      # filler 2: phi_r += r2 (in place)
                      nc.vector.tensor_tensor(out=phi[:, 4:8], in0=phi[:, 4:8],
                                              in1=rrt[:, 4:8], op=OP.add)
                      # hD = ((col<4) - u^16) * Z = [pn*(1-x1) | -bn*x2],
                      # written straight into the chunk output buffer
                      # (streamed out; host turns it into runoff)
                      nc.vector._custom_dve(FUSE_EXP3B, out=h,
                                            in0=u[:], in1=z8, s0=4.0)
                      # r' = clip(phi + hD, eps, 1)
                      nc.vector._custom_dve(FUSE_RCLIPA, out=rrt[:],
                                            in0=phi[:], in1=h, s0=EPS)

                  nc.sync.dma_start(
                      out=HO[:, ds(ci * (k_chunk * 8), k_chunk * 8)],
                      in_=hc[:])
    nc.compile()
    return nc


def _host_prepare(forcing, initial_state, raw_params, param_lower, param_upper,
                  t_total=T):
    """Derive per-core input arrays. All fp32, same op order as the device."""
    f32 = np.float32
    lo = param_lower.astype(f32)
    hi = param_upper.astype(f32)
    sg = (1.0 / (1.0 + np.exp(-raw_params.astype(np.float64))))
    phys = (lo.astype(np.float64) + (hi - lo).astype(np.float64) * sg).astype(f32)
    mw1, mw2, percrte, baserte, qbp, axv = [phys[:, i].copy() for i in range(6)]
    inv1 = (f32(1.0) / mw1).astype(f32)
    inv2 = (f32(1.0) / mw2).astype(f32)

    p_r = forcing[:, :, 0].astype(f32)    # [T, H]
    pet = forcing[:, :, 1].astype(f32)

    pn = (p_r * inv1[None, :]).astype(f32)
    ap1 = (f32(1.0) - ((pet + percrte[None, :]) * inv1[None, :])).astype(f32)
    bn = (baserte * inv2).astype(f32)
    pc12 = (percrte * inv2).astype(f32)

    s1n = np.clip(initial_state[:, 0].astype(f32) * inv1, EPS, 1.0).astype(f32)
    s2n = np.clip(initial_state[:, 1].astype(f32) * inv2, EPS, 1.0).astype(f32)

    in_maps = []
    for c in range(NCORES):
        sl = slice(c * HC, (c + 1) * HC)
        # [T, HC] -> [T, G, P] -> [P, T, G]
        def tg(a):
            return np.ascontiguousarray(
                a[:, sl].reshape(t_total, G, P).transpose(2, 0, 1))
        ZA = np.empty((P, t_total, 16), f32)
        ZA[:, :, 0:4] = tg(pn)
        ZA[:, :, 4:8] = bn[sl].reshape(G, P).T[:, None, :]
        ZA[:, :, 8:12] = tg(ap1)
        ZA[:, :, 12:16] = pc12[sl].reshape(G, P).T[:, None, :]

        def pk(a1, a2):
            out = np.empty((P, 8), f32)
            out[:, 0:4] = a1[sl].reshape(G, P).T
            out[:, 4:8] = a2[sl].reshape(G, P).T
            return out

        # exponent tile pre-divided by 16 (the exp2 range reduction)
        consts = np.concatenate([pk(s1n, s2n),
                                 pk(axv / np.float32(16.0),
                                    qbp / np.float32(16.0)),
                                 pk(mw1, mw2)], axis=1)
        in_maps.append({
            "ZA": ZA.reshape(P, t_total * 16),
            "CONSTS": consts,
        })
    return in_maps, (mw1, mw2)


_NC_CACHE = {}


def kernel(forcing, initial_state, raw_params, param_lower, param_upper):
    forcing = np.asarray(forcing)
    initial_state = np.asarray(initial_state)
    raw_params = np.asarray(raw_params)
    param_lower = np.asarray(param_lower)
    param_upper = np.asarray(param_upper)
    t_total = forcing.shape[0]
    if t_total not in _NC_CACHE:
        _NC_CACHE[t_total] = build_nc(t_total=t_total)
    nc = _NC_CACHE[t_total]
    in_maps, (mw1, mw2) = _host_prepare(forcing, initial_state, raw_params,
                                        param_lower, param_upper,
                                        t_total=t_total)
    res = run_bass_kernel_spmd(nc, in_maps, core_ids=list(range(NCORES)))
    # per-core HO: [P, T, 8]; cols g / 4+g hold hD = (pn*(1-x1), -bn*x2) of
    # HRU g*P+p; runoff = qsx + qb = p - (hD1*m1 + hD2*m2)
    p_raw = forcing[:, :, 0].astype(np.float32)    # [T, H]
    out = np.empty((t_total, H), np.float32)
    for c in range(NCORES):
        sl = slice(c * HC, (c + 1) * HC)
        ho = res.results[c]["HO"].reshape(P, t_total, 8)
        m1c = mw1[sl].reshape(G, P)
        m2c = mw2[sl].reshape(G, P)
        for g in range(G):
            hd1 = ho[:, :, g]                # [P, T]
            hd2 = ho[:, :, 4 + g]
            cols = slice(c * HC + g * P, c * HC + (g + 1) * P)
            out[:, cols] = p_raw[:, cols] - (
                hd1 * m1c[g][:, None] + hd2 * m2c[g][:, None]).T
    return out
